# revision 6
# baseline (speedup 1.0000x reference)
"""Trainium2 Bass kernel for nn_Conv2dKan (KAN-style 3x3 conv, 64->128 ch).

Math: out[b,o,l] = sum_k silu(u)*w_b + sum_{n,k} H_n(u)*(c*w_s), with u =
unfold(x) (3x3, pad 1). Linear in the basis functions, so the Hermite basis
H_0..H_7 is re-expressed in the monomial basis {u, u^2, ..., u^7} with the
basis change folded into the weights on the host; silu itself is folded in
as a degree-7 least-squares polynomial fit over the actual input values
(the silu term carries w_b ~ 1e-3 of the output scale, so the ~4e-2 fit
error lands ~1e-6 relative).  Constant terms (H_even at u=0 and the fit's
a_0) are a per-o bias added on the host after gather.

Device work per core (one batch item): x arrives pre-padded and duplicated
across both partition halves ([x|x] and [1|x] tiles); a short ACT/DVE/Pool
chain builds three fp32 plane-pair tiles [s|us], [s2|us2], [s3|us3] (s=u^2)
while the PE already streams chunk 0 (= the raw [x|x] tile, upper-half
weights zero).  Implicit GEMM: 4 chunks x 9 shifted-window taps x 5 row
tiles, PSUM-accumulated, fp32r weights x fp32r activations (~fp22 products).  Evacuation: DVE PSUM->SBUF copies staggered per row tile,
bias added on host.

Sharding: batch 8 -> one image per NeuronCore, fully data parallel.
"""

import sys

if "/opt/trn_rl_repo" not in sys.path:
    sys.path.insert(0, "/opt/trn_rl_repo")

import numpy as np

import concourse.bacc as bacc
import concourse.bass as bass
import concourse.tile as tile
from concourse import mybir
from concourse.bass_utils import run_bass_kernel_spmd

# Problem constants (hardcoded per harness contract).
B = 8
C_IN = 64
C_OUT = 128
K = 3
N_BASIS = 8
H = W = 48
HP = WP = H + 2  # padded image
L = H * W
PADN = HP * WP  # 2500
NTAPS = K * K
NCHUNK = 4
ROW_TILES = (10, 10, 10, 10, 8)
N_WARM = 8

_CACHE = {}


def _build_program():
    nc = bacc.Bacc("TRN2", target_bir_lowering=False, debug=False, num_devices=1)
    f32 = mybir.dt.float32
    f32r = mybir.dt.float32r
    f16 = mybir.dt.float16
    ACT = mybir.ActivationFunctionType

    xx_d = nc.dram_tensor("xx", [128, PADN], f32r, kind="ExternalInput").ap()
    w_d = nc.dram_tensor("w", [128, NCHUNK * NTAPS * 128], f32r, kind="ExternalInput").ap()
    o_d = nc.dram_tensor("out", [C_OUT, L], f32, kind="ExternalOutput").ap()

    CS = (0, 834, 1667, PADN)  # column slice bounds for DMA/elementwise ops

    with tile.TileContext(nc) as tc:
        with (
            tc.tile_pool(name="big", bufs=1) as wpool,
            tc.tile_pool(name="outs", bufs=3) as opool,
            tc.tile_pool(name="psum", bufs=1, space="PSUM") as ppool,
        ):
            x_sb = wpool.tile([128, PADN], f32r, tag="xx")        # [x | x]
            onex = wpool.tile([128, PADN], f32, tag="onex")       # [1 | x]
            t2 = wpool.tile([128, PADN], f32, tag="t2")           # [s | s]
            t3 = wpool.tile([128, PADN], f32, tag="t3")           # [s2 | s2]
            t23 = wpool.tile([128, PADN], f32, tag="t23")         # [s3 | s3]
            c1 = wpool.tile([128, PADN], f32r, tag="c1")          # [s | us]
            c2 = wpool.tile([128, PADN], f32r, tag="c2")          # [s2 | us2]
            c3 = wpool.tile([128, PADN], f32r, tag="c3")          # [s3 | us3]
            w_sb = wpool.tile([128, NCHUNK * NTAPS * 128], f32r)
            warm = wpool.tile([128, 512], f32r, tag="warm")

            x_f32 = x_sb.bitcast(f32)

            # ---- input DMAs (issue order per ring = priority) ----
            # sync ring: the x image slices (gate the chunk-0 stream)
            for b in range(3):
                nc.sync.dma_start(out=x_sb[:, CS[b] : CS[b + 1]], in_=xx_d[:, CS[b] : CS[b + 1]])
            # weights split across scalar (chunks 0-1) + sync (chunks 2-3)
            # rings so chunk-0 weights land ~2x sooner and the GEMM starts
            # earlier (DVE can't issue DMAs; sync is idle after the x slices).
            WB = NTAPS * 128
            for j in (0, 1):
                nc.scalar.dma_start(out=w_sb[:, j * WB : (j + 1) * WB], in_=w_d[:, j * WB : (j + 1) * WB])
            for j in (2, 3):
                nc.sync.dma_start(out=w_sb[:, j * WB : (j + 1) * WB], in_=w_d[:, j * WB : (j + 1) * WB])
            # gpsimd ring: build [1|x] on-chip (no HBM input): ones memset +
            # sliced SBUF->SBUF copies of the x upper half as xx slices land.
            nc.gpsimd.memset(warm.bitcast(f32)[:], 0.0)
            nc.gpsimd.memset(onex[0:64, :], 1.0)
            for b in range(3):
                nc.gpsimd.dma_start(
                    out=onex[64:128, CS[b] : CS[b + 1]],
                    in_=x_f32[64:128, CS[b] : CS[b + 1]],
                )

            # ---- PE pre-warm while DMAs land (HAM/pstate ramp) ----
            warm_ps = ppool.tile([128, 512], f32, tag="warm_ps")
            for _ in range(N_WARM):
                nc.tensor.matmul(warm_ps[:], warm[:, 0:128], warm[:], start=True, stop=True)

            # ---- feature planes (sliced; all overlap the matmul stream) ----
            # ACT: t2 = x^2, t3 = s^2
            for b in range(3):
                nc.scalar.activation(t2[:, CS[b] : CS[b + 1]], x_f32[:, CS[b] : CS[b + 1]], ACT.Square)
            for b in range(3):
                nc.scalar.activation(t3[:, CS[b] : CS[b + 1]], t2[:, CS[b] : CS[b + 1]], ACT.Square)
            # DVE: c1 = t2*onex = [s|us], then t23 = s^3, later evacs
            for b in range(3):
                nc.vector.tensor_mul(c1[:, CS[b] : CS[b + 1]], t2[:, CS[b] : CS[b + 1]], onex[:, CS[b] : CS[b + 1]])
            for b in range(3):
                nc.vector.tensor_mul(t23[:, CS[b] : CS[b + 1]], t2[:, CS[b] : CS[b + 1]], t3[:, CS[b] : CS[b + 1]])
            # Pool: c2 = t3*onex = [s2|us2], c3 = t23*onex = [s3|us3]
            for b in range(3):
                nc.gpsimd.tensor_mul(c2[:, CS[b] : CS[b + 1]], t3[:, CS[b] : CS[b + 1]], onex[:, CS[b] : CS[b + 1]])
            for b in range(3):
                nc.gpsimd.tensor_mul(c3[:, CS[b] : CS[b + 1]], t23[:, CS[b] : CS[b + 1]], onex[:, CS[b] : CS[b + 1]])

            # ---- implicit GEMM ----
            # chunk 0: tile-outer/tap-inner (pipelines with the sliced x DMA).
            # chunks 1-3: tap-outer/tile-inner, so 5 consecutive matmuls share
            # one lhsT (lets the codegen amortize LDWEIGHTS 1:5).
            chunks = [x_sb, c1, c2, c3]
            chunk_ims = [t.rearrange("c (h w) -> c h w", h=HP) for t in chunks]
            psums = []
            h0s = []
            h0 = 0
            for it, R in enumerate(ROW_TILES):
                psums.append(ppool.tile([128, R * W], f32, name=f"ps{h0}", tag=f"ps{it}"))
                h0s.append(h0)
                h0 += R
            out_rings = (nc.sync, nc.gpsimd, nc.sync, nc.gpsimd)

            def mm(j, t9, it, R):
                lhsT = w_sb[:, (j * NTAPS + t9) * 128 : (j * NTAPS + t9 + 1) * 128]
                dh, dw = t9 // K - 1, t9 % K - 1
                r0 = h0s[it] + dh + 1
                rhs = chunk_ims[j][:, r0 : r0 + R, dw + 1 : dw + 1 + W]
                nc.tensor.matmul(
                    psums[it][:],
                    lhsT,
                    rhs,
                    start=(j == 0 and t9 == 0),
                    stop=(j == NCHUNK - 1 and t9 == NTAPS - 1),
                )

            def evac(it, R):
                # staggered evacuation: DVE PSUM->SBUF, then DMA out
                h0 = h0s[it]
                o_sb = opool.tile([C_OUT, R * W], f32, tag="osb")
                if it < len(ROW_TILES) - 1:
                    nc.vector.tensor_copy(o_sb[:], psums[it][:])
                    out_rings[it].dma_start(out=o_d[:, h0 * W : (h0 + R) * W], in_=o_sb[:])
                else:
                    # last tile: halve so the final DMA starts sooner
                    hn = R * W // 2
                    for hh, eng in ((0, nc.sync), (1, nc.gpsimd)):
                        nc.vector.tensor_copy(
                            o_sb[:, hh * hn : (hh + 1) * hn],
                            psums[it][:, hh * hn : (hh + 1) * hn],
                        )
                        eng.dma_start(
                            out=o_d[:, h0 * W + hh * hn : h0 * W + (hh + 1) * hn],
                            in_=o_sb[:, hh * hn : (hh + 1) * hn],
                        )

            for it, R in enumerate(ROW_TILES):
                for t9 in range(NTAPS):
                    mm(0, t9, it, R)
            for j in (1, 2, 3):
                for t9 in range(NTAPS):
                    for it, R in enumerate(ROW_TILES):
                        mm(j, t9, it, R)
                        if j == NCHUNK - 1 and t9 == NTAPS - 1:
                            evac(it, R)

    nc.compile()
    return nc


def _host_prep(x, w_b, w_s, c):
    """Fold Hermite->monomial basis change, w_s, and a degree-7 polynomial
    fit of silu into the weights (fp64 host math)."""
    wb = w_b[..., 0].astype(np.float64)          # (O, 576)
    cw = (c[..., 0] * w_s[None, ..., 0]).astype(np.float64)  # (N, O, 576)

    # monomial weights for planes u^1..u^7 (+ constant -> bias)
    wm = np.zeros((8, C_OUT, C_IN * NTAPS), np.float64)
    wm[1] = 2 * cw[1] - 12 * cw[3] + 120 * cw[5] - 1680 * cw[7]
    wm[2] = 2 * cw[2] - 48 * cw[4] + 720 * cw[6]
    wm[3] = 8 * cw[3] - 160 * cw[5] + 3360 * cw[7]
    wm[4] = 16 * cw[4] - 480 * cw[6]
    wm[5] = 32 * cw[5] - 1344 * cw[7]
    wm[6] = 64 * cw[6]
    wm[7] = 128 * cw[7]
    bias = (cw[0] - 2 * cw[2] + 12 * cw[4] - 120 * cw[6]).sum(axis=1)  # (O,)

    # degree-7 LS fit of silu over the actual input values (+ Chebyshev
    # nodes over the input range for tail control), folded into wm/bias
    xs = np.asarray(x, np.float64).ravel()
    m = np.abs(xs).max() * 1.02
    nodes = m * np.cos(np.pi * (np.arange(2000) + 0.5) / 2000)
    fitx = np.concatenate([xs[::37], nodes, nodes, nodes])
    A = np.vander(fitx, 8, increasing=True)
    coef, *_ = np.linalg.lstsq(A, fitx / (1 + np.exp(-fitx)), rcond=None)
    for f in range(1, 8):
        wm[f] += coef[f] * wb
    bias = bias + coef[0] * wb.sum(axis=1)

    # lhsT pack: [k_part=128, chunk=4, tap=9, o=128] fp16
    # chunk j, k_part = 64*half + c_in -> plane: j==0: (u | zero),
    # j>=1: (u^{2j} | u^{2j+1})
    wl = np.zeros((128, NCHUNK, NTAPS, C_OUT), np.float32)
    cidx = np.arange(C_IN)
    plane_of = {(0, 0): 1, (1, 0): 2, (1, 1): 3, (2, 0): 4, (2, 1): 5, (3, 0): 6, (3, 1): 7}
    for j in range(NCHUNK):
        for half in range(2):
            f = plane_of.get((j, half))
            if f is None:
                continue
            for t in range(NTAPS):
                wl[64 * half : 64 * (half + 1), j, t, :] = (
                    wm[f][:, cidx * NTAPS + t].T.astype(np.float32)
                )
    return wl.reshape(128, NCHUNK * NTAPS * 128), bias.astype(np.float32)


def _prep_in_maps(x, w_b, w_s, c):
    wl, bias = _host_prep(x, w_b, w_s, c)
    xi = np.asarray(x, np.float32)
    xp = np.zeros((B, C_IN, HP, WP), np.float32)
    xp[:, :, 1 : 1 + H, 1 : 1 + W] = xi
    xp = xp.reshape(B, C_IN, PADN)
    in_maps = []
    for i in range(B):
        xx = np.concatenate([xp[i], xp[i]], axis=0)        # [x | x]
        in_maps.append({"xx": xx, "w": wl})
    return in_maps, bias


def kernel(x, w_b, w_s, c):
    if "nc" not in _CACHE:
        _CACHE["nc"] = _build_program()
    nc = _CACHE["nc"]

    in_maps, bias = _prep_in_maps(x, w_b, w_s, c)
    res = run_bass_kernel_spmd(nc, in_maps, core_ids=list(range(B)))
    out = np.stack([res.results[i]["out"] for i in range(B)], axis=0)
    out += bias[None, :, None]
    return out.reshape(B, C_OUT, H, W)



# revision 7
# speedup vs baseline: 1.0350x; 1.0350x over previous
"""Trainium2 Bass kernel for nn_Conv2dKan (KAN-style 3x3 conv, 64->128 ch).

Math: out[b,o,l] = sum_k silu(u)*w_b + sum_{n,k} H_n(u)*(c*w_s), with u =
unfold(x) (3x3, pad 1). Linear in the basis functions, so the Hermite basis
H_0..H_7 is re-expressed in the monomial basis {u, u^2, ..., u^7} with the
basis change folded into the weights on the host; silu itself is folded in
as a degree-7 least-squares polynomial fit over the actual input values.
Constant terms are a per-o bias added on the host after gather.

Device work per core (one batch item): x arrives pre-padded as a [64, 2500]
tile (fine-sliced DMAs so the first row tile lands early); chunk 0 of the
implicit GEMM runs K=64 matmuls straight off it while [x|x] / [1|x] tiles
are built on-chip (SBUF->SBUF DMA copies + memset) to feed the short
ACT/DVE/Pool chain producing the plane pairs [u^2|u^3], [u^4|u^5],
[u^6|u^7].  Implicit GEMM: chunk 0 (K=64) + 3 chunks (K=128) x 9 shifted
window taps x 5 row tiles, PSUM-accumulated in fp32, fp32r x fp32r.
Evacuation staggered per row tile (DVE PSUM->SBUF copy, then DMA out).

Sharding: batch 8 -> one image per NeuronCore, fully data parallel.
"""

import sys

if "/opt/trn_rl_repo" not in sys.path:
    sys.path.insert(0, "/opt/trn_rl_repo")

import numpy as np

import concourse.bacc as bacc
import concourse.bass as bass
import concourse.tile as tile
from concourse import mybir
from concourse.bass_utils import run_bass_kernel_spmd

# Problem constants (hardcoded per harness contract).
B = 8
C_IN = 64
C_OUT = 128
K = 3
N_BASIS = 8
H = W = 48
HP = WP = H + 2  # padded image
L = H * W
PADN = HP * WP  # 2500
NTAPS = K * K
NCHUNK = 4
ROW_TILES = (10, 10, 10, 10, 8)
N_WARM = 10

_CACHE = {}


def _build_program():
    nc = bacc.Bacc("TRN2", target_bir_lowering=False, debug=False, num_devices=1)
    f32 = mybir.dt.float32
    f32r = mybir.dt.float32r
    ACT = mybir.ActivationFunctionType

    x_d = nc.dram_tensor("x64", [64, PADN], f32r, kind="ExternalInput").ap()
    w0_d = nc.dram_tensor("w0", [64, NTAPS * 128], f32r, kind="ExternalInput").ap()
    w_d = nc.dram_tensor("w", [128, 3 * NTAPS * 128], f32r, kind="ExternalInput").ap()
    o_d = nc.dram_tensor("out", [C_OUT, L], f32, kind="ExternalOutput").ap()

    # x DMA slices: boundaries aligned so row tile r (rows 10r..10r+R+1,
    # i.e. cols < (10r+R+2)*50) is covered by the first slices.
    XS = (0, 625, 1250, 1875, PADN)
    CS = (0, 834, 1667, PADN)  # slice bounds for elementwise / copies

    with tile.TileContext(nc) as tc:
        with (
            tc.tile_pool(name="big", bufs=1) as wpool,
            tc.tile_pool(name="outs", bufs=3) as opool,
            tc.tile_pool(name="psum", bufs=1, space="PSUM") as ppool,
        ):
            x64 = wpool.tile([64, PADN], f32r, tag="x64")
            xx = wpool.tile([128, PADN], f32, tag="xx")          # [x | x]
            onex = wpool.tile([128, PADN], f32, tag="onex")      # [1 | x]
            t2 = wpool.tile([128, PADN], f32, tag="t2")          # [s | s]
            t3 = wpool.tile([128, PADN], f32, tag="t3")          # [s2 | s2]
            t23 = wpool.tile([128, PADN], f32, tag="t23")        # [s3 | s3]
            c1 = wpool.tile([128, PADN], f32r, tag="c1")         # [s | us]
            c2 = wpool.tile([128, PADN], f32r, tag="c2")         # [s2 | us2]
            c3 = wpool.tile([128, PADN], f32r, tag="c3")         # [s3 | us3]
            w0_sb = wpool.tile([64, NTAPS * 128], f32r, tag="w0")
            w_sb = wpool.tile([128, 3 * NTAPS * 128], f32r)
            warm = wpool.tile([128, 256], f32r, tag="warm")

            x_f32 = x64.bitcast(f32)

            # ---- input DMAs (fine-sliced; each dma_start gets its own
            # hardware queue, so slicing shortens the critical landing) ----
            # sync ring: x slices (gate chunk-0), then w chunks 2-3.
            for b in range(4):
                nc.sync.dma_start(out=x64[:, XS[b] : XS[b + 1]], in_=x_d[:, XS[b] : XS[b + 1]])
            # scalar ring: chunk-0 weights in 3-tap pieces, then chunk 1.
            WB = NTAPS * 128
            for p in range(3):
                nc.scalar.dma_start(
                    out=w0_sb[:, p * 384 : (p + 1) * 384], in_=w0_d[:, p * 384 : (p + 1) * 384]
                )
            nc.scalar.dma_start(out=w_sb[:, 0:WB], in_=w_d[:, 0:WB])
            nc.sync.dma_start(out=w_sb[:, WB : 2 * WB], in_=w_d[:, WB : 2 * WB])
            nc.sync.dma_start(out=w_sb[:, 2 * WB : 3 * WB], in_=w_d[:, 2 * WB : 3 * WB])

            # gpsimd ring: PE warm tile, then build [x|x] and [1|x] on-chip.
            nc.gpsimd.memset(warm.bitcast(f32)[:], 0.0)
            nc.gpsimd.memset(onex[0:64, :], 1.0)
            for b in range(3):
                nc.gpsimd.dma_start(out=xx[0:64, CS[b] : CS[b + 1]], in_=x_f32[:, CS[b] : CS[b + 1]])
            for b in range(3):
                nc.gpsimd.dma_start(out=xx[64:128, CS[b] : CS[b + 1]], in_=x_f32[:, CS[b] : CS[b + 1]])
            for b in range(3):
                nc.gpsimd.dma_start(out=onex[64:128, CS[b] : CS[b + 1]], in_=x_f32[:, CS[b] : CS[b + 1]])

            # ---- PE pre-warm while DMAs land (HAM/pstate ramp) ----
            warm_ps = ppool.tile([128, 256], f32, tag="warm_ps")
            for _ in range(N_WARM):
                nc.tensor.matmul(warm_ps[:], warm[:, 0:128], warm[:], start=True, stop=True)

            # ---- feature planes (sliced; overlap the chunk-0 stream) ----
            # ACT: t2 = x^2, t3 = s^2
            for b in range(3):
                nc.scalar.activation(t2[:, CS[b] : CS[b + 1]], xx[:, CS[b] : CS[b + 1]], ACT.Square)
            for b in range(3):
                nc.scalar.activation(t3[:, CS[b] : CS[b + 1]], t2[:, CS[b] : CS[b + 1]], ACT.Square)
            # DVE: c1 = t2*onex = [s|us], then t23 = s^3
            for b in range(3):
                nc.vector.tensor_mul(c1[:, CS[b] : CS[b + 1]], t2[:, CS[b] : CS[b + 1]], onex[:, CS[b] : CS[b + 1]])
            for b in range(3):
                nc.vector.tensor_mul(t23[:, CS[b] : CS[b + 1]], t2[:, CS[b] : CS[b + 1]], t3[:, CS[b] : CS[b + 1]])
            # Pool: c2 = t3*onex = [s2|us2], c3 = t23*onex = [s3|us3]
            for b in range(3):
                nc.gpsimd.tensor_mul(c2[:, CS[b] : CS[b + 1]], t3[:, CS[b] : CS[b + 1]], onex[:, CS[b] : CS[b + 1]])
            for b in range(3):
                nc.gpsimd.tensor_mul(c3[:, CS[b] : CS[b + 1]], t23[:, CS[b] : CS[b + 1]], onex[:, CS[b] : CS[b + 1]])

            # ---- implicit GEMM: chunk-outer, tile-mid, tap-inner ----
            x64_im = x64.rearrange("c (h w) -> c h w", h=HP)
            chunk_ims = [t.rearrange("c (h w) -> c h w", h=HP) for t in (c1, c2, c3)]
            psums = []
            h0s = []
            h0 = 0
            for it, R in enumerate(ROW_TILES):
                psums.append(ppool.tile([128, R * W], f32, name=f"ps{h0}", tag=f"ps{it}"))
                h0s.append(h0)
                h0 += R
            out_rings = (nc.sync, nc.gpsimd, nc.sync, nc.gpsimd)

            # chunk 0: K=64 matmuls straight off the half-partition x tile
            for it, R in enumerate(ROW_TILES):
                h0 = h0s[it]
                for t9 in range(NTAPS):
                    dh, dw = t9 // K - 1, t9 % K - 1
                    r0 = h0 + dh + 1
                    nc.tensor.matmul(
                        psums[it][:],
                        w0_sb[:, t9 * 128 : (t9 + 1) * 128],
                        x64_im[:, r0 : r0 + R, dw + 1 : dw + 1 + W],
                        start=(t9 == 0),
                        stop=False,
                    )
            # chunks 1-3 (K=128), staggered per-tile evacuation on the last
            for jj, im in enumerate(chunk_ims):
                for it, R in enumerate(ROW_TILES):
                    h0 = h0s[it]
                    for t9 in range(NTAPS):
                        dh, dw = t9 // K - 1, t9 % K - 1
                        r0 = h0 + dh + 1
                        nc.tensor.matmul(
                            psums[it][:],
                            w_sb[:, (jj * NTAPS + t9) * 128 : (jj * NTAPS + t9 + 1) * 128],
                            im[:, r0 : r0 + R, dw + 1 : dw + 1 + W],
                            start=False,
                            stop=(jj == 2 and t9 == NTAPS - 1),
                        )
                    if jj == 2:
                        # staggered evacuation: DVE PSUM->SBUF, then DMA out
                        o_sb = opool.tile([C_OUT, R * W], f32, tag="osb")
                        if it < len(ROW_TILES) - 1:
                            nc.vector.tensor_copy(o_sb[:], psums[it][:])
                            out_rings[it].dma_start(
                                out=o_d[:, h0 * W : (h0 + R) * W], in_=o_sb[:]
                            )
                        else:
                            # last tile: halve so the final DMA starts sooner
                            hn = R * W // 2
                            for hh, eng in ((0, nc.sync), (1, nc.gpsimd)):
                                nc.vector.tensor_copy(
                                    o_sb[:, hh * hn : (hh + 1) * hn],
                                    psums[it][:, hh * hn : (hh + 1) * hn],
                                )
                                eng.dma_start(
                                    out=o_d[:, h0 * W + hh * hn : h0 * W + (hh + 1) * hn],
                                    in_=o_sb[:, hh * hn : (hh + 1) * hn],
                                )

    nc.compile()
    return nc


def _host_prep(x, w_b, w_s, c):
    """Fold Hermite->monomial basis change, w_s, and a degree-7 polynomial
    fit of silu into the weights (fp64 host math)."""
    wb = w_b[..., 0].astype(np.float64)          # (O, 576)
    cw = (c[..., 0] * w_s[None, ..., 0]).astype(np.float64)  # (N, O, 576)

    # monomial weights for planes u^1..u^7 (+ constant -> bias)
    wm = np.zeros((8, C_OUT, C_IN * NTAPS), np.float64)
    wm[1] = 2 * cw[1] - 12 * cw[3] + 120 * cw[5] - 1680 * cw[7]
    wm[2] = 2 * cw[2] - 48 * cw[4] + 720 * cw[6]
    wm[3] = 8 * cw[3] - 160 * cw[5] + 3360 * cw[7]
    wm[4] = 16 * cw[4] - 480 * cw[6]
    wm[5] = 32 * cw[5] - 1344 * cw[7]
    wm[6] = 64 * cw[6]
    wm[7] = 128 * cw[7]
    bias = (cw[0] - 2 * cw[2] + 12 * cw[4] - 120 * cw[6]).sum(axis=1)  # (O,)

    # degree-7 LS fit of silu over the actual input values (+ Chebyshev
    # nodes over the input range for tail control), folded into wm/bias
    xs = np.asarray(x, np.float64).ravel()
    m = np.abs(xs).max() * 1.02
    nodes = m * np.cos(np.pi * (np.arange(2000) + 0.5) / 2000)
    fitx = np.concatenate([xs[::37], nodes, nodes, nodes])
    A = np.vander(fitx, 8, increasing=True)
    coef, *_ = np.linalg.lstsq(A, fitx / (1 + np.exp(-fitx)), rcond=None)
    for f in range(1, 8):
        wm[f] += coef[f] * wb
    bias = bias + coef[0] * wb.sum(axis=1)

    # chunk 0 (plane u, K=64): [k=64, tap=9, o=128]
    cidx = np.arange(C_IN)
    w0 = np.zeros((64, NTAPS, C_OUT), np.float32)
    for t in range(NTAPS):
        w0[:, t, :] = wm[1][:, cidx * NTAPS + t].T.astype(np.float32)
    # chunks 1-3: [k_part=128, chunk=3, tap=9, o=128]
    # chunk j, k_part = 64*half + c_in -> plane u^{2j+2+half}
    wl = np.zeros((128, 3, NTAPS, C_OUT), np.float32)
    for j in range(3):
        for half in range(2):
            f = 2 * j + 2 + half
            for t in range(NTAPS):
                wl[64 * half : 64 * (half + 1), j, t, :] = (
                    wm[f][:, cidx * NTAPS + t].T.astype(np.float32)
                )
    return (
        w0.reshape(64, NTAPS * 128),
        wl.reshape(128, 3 * NTAPS * 128),
        bias.astype(np.float32),
    )


def _prep_in_maps(x, w_b, w_s, c):
    w0, wl, bias = _host_prep(x, w_b, w_s, c)
    xi = np.asarray(x, np.float32)
    xp = np.zeros((B, C_IN, HP, WP), np.float32)
    xp[:, :, 1 : 1 + H, 1 : 1 + W] = xi
    xp = xp.reshape(B, C_IN, PADN)
    in_maps = []
    for i in range(B):
        in_maps.append({"x64": xp[i], "w0": w0, "w": wl})
    return in_maps, bias


def kernel(x, w_b, w_s, c):
    if "nc" not in _CACHE:
        _CACHE["nc"] = _build_program()
    nc = _CACHE["nc"]

    in_maps, bias = _prep_in_maps(x, w_b, w_s, c)
    res = run_bass_kernel_spmd(nc, in_maps, core_ids=list(range(B)))
    out = np.stack([res.results[i]["out"] for i in range(B)], axis=0)
    out += bias[None, :, None]
    return out.reshape(B, C_OUT, H, W)


# revision 11
# speedup vs baseline: 1.1434x; 1.1047x over previous
"""Trainium2 Bass kernel for nn_Conv2dKan (KAN-style 3x3 conv, 64->128 ch).

Math: out[b,o,l] = sum_k silu(u)*w_b + sum_{n,k} H_n(u)*(c*w_s), with u =
unfold(x) (3x3, pad 1). Linear in the basis functions, so the Hermite basis
H_0..H_7 is re-expressed in the monomial basis {u, u^2, ..., u^7} with the
basis change folded into the weights on the host; silu itself is folded in
as a degree-7 least-squares polynomial fit over the actual input values.
Constant terms are a per-o bias added on the host after gather.

Device work per core (one batch item): x arrives pre-padded as a [64, 2500]
tile (fine-sliced DMAs so the first row tile lands early); chunk 0 of the
implicit GEMM runs K=64 matmuls straight off it while [x|x] / [1|x] tiles
are built on-chip (SBUF->SBUF DMA copies + memset) to feed the short
ACT/DVE/Pool chain producing the plane pairs [u^2|u^3], [u^4|u^5],
[u^6|u^7].  Implicit GEMM: chunk 0 (K=64) + 3 chunks (K=128) x 9 shifted
window taps x 5 row tiles, PSUM-accumulated in fp32, fp32r x fp32r.
Evacuation staggered per row tile (DVE PSUM->SBUF copy, then DMA out).

Sharding: batch 8 -> one image per NeuronCore, fully data parallel.
"""

import sys

if "/opt/trn_rl_repo" not in sys.path:
    sys.path.insert(0, "/opt/trn_rl_repo")

import numpy as np

import concourse.bacc as bacc
import concourse.bass as bass
import concourse.tile as tile
from concourse import mybir
from concourse.bass_utils import run_bass_kernel_spmd

# Problem constants (hardcoded per harness contract).
B = 8
C_IN = 64
C_OUT = 128
K = 3
N_BASIS = 8
H = W = 48
HP = WP = H + 2  # padded image
L = H * W
PADN = HP * WP  # 2500
NTAPS = K * K
NCHUNK = 4
ROW_TILES = (10, 10, 10, 10, 8)
N_WARM = 10

_CACHE = {}


def _build_program():
    nc = bacc.Bacc("TRN2", target_bir_lowering=False, debug=False, num_devices=1)
    f32 = mybir.dt.float32
    f32r = mybir.dt.float32r
    ACT = mybir.ActivationFunctionType

    xx_d = nc.dram_tensor("xx", [128, PADN], f32r, kind="ExternalInput").ap()
    w0_d = nc.dram_tensor("w0", [64, NTAPS * 128], f32r, kind="ExternalInput").ap()
    w_d = nc.dram_tensor("w", [128, 3 * NTAPS * 128], f32r, kind="ExternalInput").ap()
    o_d = nc.dram_tensor("out", [C_OUT, L], f32, kind="ExternalOutput").ap()

    # x DMA slices: boundaries aligned so row tile r (rows 10r..10r+R+1,
    # i.e. cols < (10r+R+2)*50) is covered by the first slices.
    XS = (625, 1250, 1875, PADN)
    CS = (0, 834, 1667, PADN)  # slice bounds for elementwise / copies

    with tile.TileContext(nc) as tc:
        with (
            tc.tile_pool(name="big", bufs=1) as wpool,
            tc.tile_pool(name="outs", bufs=3) as opool,
            tc.tile_pool(name="psum", bufs=1, space="PSUM") as ppool,
        ):
            x_sb = wpool.tile([128, PADN], f32r, tag="xx")        # [x | x]
            onex = wpool.tile([128, PADN], f32, tag="onex")      # [1 | x]
            t2 = wpool.tile([128, PADN], f32, tag="t2")          # [s | s]
            t3 = wpool.tile([128, PADN], f32, tag="t3")          # [s2 | s2]
            t23 = wpool.tile([128, PADN], f32, tag="t23")        # [s3 | s3]
            c1 = wpool.tile([128, PADN], f32r, tag="c1")         # [s | us]
            c2 = wpool.tile([128, PADN], f32r, tag="c2")         # [s2 | us2]
            c3 = wpool.tile([128, PADN], f32r, tag="c3")         # [s3 | us3]
            w0_sb = wpool.tile([128, NTAPS * 128], f32r, tag="w0")
            w_sb = wpool.tile([128, 3 * NTAPS * 128], f32r)
            warm = wpool.tile([128, 256], f32r, tag="warm")

            x_f32 = x_sb.bitcast(f32)

            # ---- input DMAs (fine-sliced; each dma_start gets its own
            # hardware queue, so slicing shortens the critical landing) ----
            # sync ring: x slices (gate chunk-0), then w chunks 2-3.
            # First 625 cols split by partition halves so row tile 0's data
            # lands fastest.
            nc.sync.dma_start(out=x_sb[0:64, 0:625], in_=xx_d[0:64, 0:625])
            nc.sync.dma_start(out=x_sb[64:128, 0:625], in_=xx_d[64:128, 0:625])
            for b in range(3):
                nc.sync.dma_start(out=x_sb[:, XS[b] : XS[b + 1]], in_=xx_d[:, XS[b] : XS[b + 1]])
            # scalar ring: chunk-0 weights in 3-tap pieces (lower half only;
            # the zero upper half is memset on-device), then chunk 1.
            WB = NTAPS * 128
            for p in range(3):
                nc.scalar.dma_start(
                    out=w0_sb[0:64, p * 384 : (p + 1) * 384], in_=w0_d[:, p * 384 : (p + 1) * 384]
                )
            nc.scalar.dma_start(out=w_sb[:, 0:WB], in_=w_d[:, 0:WB])
            nc.sync.dma_start(out=w_sb[:, WB : 2 * WB], in_=w_d[:, WB : 2 * WB])
            nc.sync.dma_start(out=w_sb[:, 2 * WB : 3 * WB], in_=w_d[:, 2 * WB : 3 * WB])

            # gpsimd ring: PE warm tile, chunk-0 upper weights = 0, then
            # build [1|x] on-chip ([x|x]'s upper half is the copy source).
            nc.gpsimd.memset(warm.bitcast(f32)[:], 0.0)
            nc.gpsimd.memset(w0_sb.bitcast(f32)[64:128, :], 0.0)
            nc.gpsimd.memset(onex[0:64, :], 1.0)
            for b in range(3):
                nc.gpsimd.dma_start(
                    out=onex[64:128, CS[b] : CS[b + 1]],
                    in_=x_f32[64:128, CS[b] : CS[b + 1]],
                )

            # ---- PE pre-warm while DMAs land (HAM/pstate ramp) ----
            warm_ps = ppool.tile([128, 256], f32, tag="warm_ps")
            for _ in range(N_WARM):
                nc.tensor.matmul(warm_ps[:], warm[:, 0:128], warm[:], start=True, stop=True)

            # ---- feature planes (sliced; overlap the chunk-0 stream) ----
            # ACT: t2 = x^2, t3 = s^2
            for b in range(3):
                nc.scalar.activation(t2[:, CS[b] : CS[b + 1]], x_f32[:, CS[b] : CS[b + 1]], ACT.Square)
            for b in range(3):
                nc.scalar.activation(t3[:, CS[b] : CS[b + 1]], t2[:, CS[b] : CS[b + 1]], ACT.Square)
            # DVE: c1 = t2*onex = [s|us], then t23 = s^3
            for b in range(3):
                nc.vector.tensor_mul(c1[:, CS[b] : CS[b + 1]], t2[:, CS[b] : CS[b + 1]], onex[:, CS[b] : CS[b + 1]])
            for b in range(3):
                nc.vector.tensor_mul(t23[:, CS[b] : CS[b + 1]], t2[:, CS[b] : CS[b + 1]], t3[:, CS[b] : CS[b + 1]])
            # Pool: c2 = t3*onex = [s2|us2], c3 = t23*onex = [s3|us3]
            for b in range(3):
                nc.gpsimd.tensor_mul(c2[:, CS[b] : CS[b + 1]], t3[:, CS[b] : CS[b + 1]], onex[:, CS[b] : CS[b + 1]])
            for b in range(3):
                nc.gpsimd.tensor_mul(c3[:, CS[b] : CS[b + 1]], t23[:, CS[b] : CS[b + 1]], onex[:, CS[b] : CS[b + 1]])

            # ---- implicit GEMM: chunk-outer, tile-mid, tap-inner ----
            x_im = x_sb.rearrange("c (h w) -> c h w", h=HP)
            chunk_ims = [t.rearrange("c (h w) -> c h w", h=HP) for t in (c1, c2, c3)]
            psums = []
            h0s = []
            h0 = 0
            for it, R in enumerate(ROW_TILES):
                psums.append(ppool.tile([128, R * W], f32, name=f"ps{h0}", tag=f"ps{it}"))
                h0s.append(h0)
                h0 += R
            out_rings = (nc.sync, nc.gpsimd, nc.sync, nc.gpsimd)

            # chunk 0: [x|x] tile, upper-half weights zero
            for it, R in enumerate(ROW_TILES):
                h0 = h0s[it]
                for t9 in range(NTAPS):
                    dh, dw = t9 // K - 1, t9 % K - 1
                    r0 = h0 + dh + 1
                    nc.tensor.matmul(
                        psums[it][:],
                        w0_sb[:, t9 * 128 : (t9 + 1) * 128],
                        x_im[:, r0 : r0 + R, dw + 1 : dw + 1 + W],
                        start=(t9 == 0),
                        stop=False,
                    )
            # chunks 1-3 (K=128), staggered per-tile evacuation on the last
            for jj, im in enumerate(chunk_ims):
                for it, R in enumerate(ROW_TILES):
                    h0 = h0s[it]
                    for t9 in range(NTAPS):
                        dh, dw = t9 // K - 1, t9 % K - 1
                        r0 = h0 + dh + 1
                        nc.tensor.matmul(
                            psums[it][:],
                            w_sb[:, (jj * NTAPS + t9) * 128 : (jj * NTAPS + t9 + 1) * 128],
                            im[:, r0 : r0 + R, dw + 1 : dw + 1 + W],
                            start=False,
                            stop=(jj == 2 and t9 == NTAPS - 1),
                        )
                    if jj == 2:
                        # staggered evacuation: DVE PSUM->SBUF, then DMA out
                        o_sb = opool.tile([C_OUT, R * W], f32, tag="osb")
                        if it < len(ROW_TILES) - 1:
                            nc.vector.tensor_copy(o_sb[:], psums[it][:])
                            out_rings[it].dma_start(
                                out=o_d[:, h0 * W : (h0 + R) * W], in_=o_sb[:]
                            )
                        else:
                            # last tile: halve so the final DMA starts sooner
                            hn = R * W // 2
                            for hh, eng in ((0, nc.sync), (1, nc.gpsimd)):
                                nc.vector.tensor_copy(
                                    o_sb[:, hh * hn : (hh + 1) * hn],
                                    psums[it][:, hh * hn : (hh + 1) * hn],
                                )
                                eng.dma_start(
                                    out=o_d[:, h0 * W + hh * hn : h0 * W + (hh + 1) * hn],
                                    in_=o_sb[:, hh * hn : (hh + 1) * hn],
                                )

    nc.compile()
    return nc


def _host_prep(x, w_b, w_s, c):
    """Fold Hermite->monomial basis change, w_s, and a degree-7 polynomial
    fit of silu into the weights (fp64 host math)."""
    wb = w_b[..., 0].astype(np.float64)          # (O, 576)
    cw = (c[..., 0] * w_s[None, ..., 0]).astype(np.float64)  # (N, O, 576)

    # monomial weights for planes u^1..u^7 (+ constant -> bias)
    wm = np.zeros((8, C_OUT, C_IN * NTAPS), np.float64)
    wm[1] = 2 * cw[1] - 12 * cw[3] + 120 * cw[5] - 1680 * cw[7]
    wm[2] = 2 * cw[2] - 48 * cw[4] + 720 * cw[6]
    wm[3] = 8 * cw[3] - 160 * cw[5] + 3360 * cw[7]
    wm[4] = 16 * cw[4] - 480 * cw[6]
    wm[5] = 32 * cw[5] - 1344 * cw[7]
    wm[6] = 64 * cw[6]
    wm[7] = 128 * cw[7]
    bias = (cw[0] - 2 * cw[2] + 12 * cw[4] - 120 * cw[6]).sum(axis=1)  # (O,)

    # degree-7 LS fit of silu over the actual input values (+ Chebyshev
    # nodes over the input range for tail control), folded into wm/bias
    xs = np.asarray(x, np.float64).ravel()
    m = np.abs(xs).max() * 1.02
    nodes = m * np.cos(np.pi * (np.arange(2000) + 0.5) / 2000)
    fitx = np.concatenate([xs[::37], nodes, nodes, nodes])
    A = np.vander(fitx, 8, increasing=True)
    coef, *_ = np.linalg.lstsq(A, fitx / (1 + np.exp(-fitx)), rcond=None)
    for f in range(1, 8):
        wm[f] += coef[f] * wb
    bias = bias + coef[0] * wb.sum(axis=1)

    # chunk 0 (plane u, K=64): [k=64, tap=9, o=128]
    cidx = np.arange(C_IN)
    w0 = np.zeros((64, NTAPS, C_OUT), np.float32)
    for t in range(NTAPS):
        w0[:, t, :] = wm[1][:, cidx * NTAPS + t].T.astype(np.float32)
    # chunks 1-3: [k_part=128, chunk=3, tap=9, o=128]
    # chunk j, k_part = 64*half + c_in -> plane u^{2j+2+half}
    wl = np.zeros((128, 3, NTAPS, C_OUT), np.float32)
    for j in range(3):
        for half in range(2):
            f = 2 * j + 2 + half
            for t in range(NTAPS):
                wl[64 * half : 64 * (half + 1), j, t, :] = (
                    wm[f][:, cidx * NTAPS + t].T.astype(np.float32)
                )
    return (
        w0.reshape(64, NTAPS * 128),
        wl.reshape(128, 3 * NTAPS * 128),
        bias.astype(np.float32),
    )


def _prep_in_maps(x, w_b, w_s, c):
    w0, wl, bias = _host_prep(x, w_b, w_s, c)
    xi = np.asarray(x, np.float32)
    xp = np.zeros((B, C_IN, HP, WP), np.float32)
    xp[:, :, 1 : 1 + H, 1 : 1 + W] = xi
    xp = xp.reshape(B, C_IN, PADN)
    in_maps = []
    for i in range(B):
        xx = np.concatenate([xp[i], xp[i]], axis=0)        # [x | x]
        in_maps.append({"xx": xx, "w0": w0, "w": wl})
    return in_maps, bias


def kernel(x, w_b, w_s, c):
    if "nc" not in _CACHE:
        _CACHE["nc"] = _build_program()
    nc = _CACHE["nc"]

    in_maps, bias = _prep_in_maps(x, w_b, w_s, c)
    res = run_bass_kernel_spmd(nc, in_maps, core_ids=list(range(B)))
    out = np.stack([res.results[i]["out"] for i in range(B)], axis=0)
    out += bias[None, :, None]
    return out.reshape(B, C_OUT, H, W)


# revision 15
# speedup vs baseline: 1.1646x; 1.0185x over previous
"""Trainium2 Bass kernel for nn_Conv2dKan (KAN-style 3x3 conv, 64->128 ch).

Math: out[b,o,l] = sum_k silu(u)*w_b + sum_{n,k} H_n(u)*(c*w_s), with u =
unfold(x) (3x3, pad 1). Linear in the basis functions, so the Hermite basis
H_0..H_7 is re-expressed in the monomial basis {u, u^2, ..., u^7} with the
basis change folded into the weights on the host; silu itself is folded in
as a degree-7 least-squares polynomial fit over the actual input values.
Constant terms are a per-o bias added on the host after gather.

Device work per core (one batch item): x arrives pre-padded as a [64, 2500]
tile (fine-sliced DMAs so the first row tile lands early); chunk 0 of the
implicit GEMM runs K=64 matmuls straight off it while [x|x] / [1|x] tiles
are built on-chip (SBUF->SBUF DMA copies + memset) to feed the short
ACT/DVE/Pool chain producing the plane pairs [u^2|u^3], [u^4|u^5],
[u^6|u^7].  Implicit GEMM: chunk 0 (K=64) + 3 chunks (K=128) x 9 shifted
window taps x 5 row tiles, PSUM-accumulated in fp32, fp32r x fp32r.
Evacuation staggered per row tile (DVE PSUM->SBUF copy, then DMA out).

Sharding: batch 8 -> one image per NeuronCore, fully data parallel.
"""

import sys

if "/opt/trn_rl_repo" not in sys.path:
    sys.path.insert(0, "/opt/trn_rl_repo")

import numpy as np

import concourse.bacc as bacc
import concourse.bass as bass
import concourse.tile as tile
from concourse import mybir
from concourse.bass_utils import run_bass_kernel_spmd

# Problem constants (hardcoded per harness contract).
B = 8
C_IN = 64
C_OUT = 128
K = 3
N_BASIS = 8
H = W = 48
HP = WP = H + 2  # padded image
L = H * W
PADN = HP * WP  # 2500
NTAPS = K * K
NCHUNK = 4
ROW_TILES = (10, 10, 10, 10, 8)
N_WARM = 12

_CACHE = {}


def _build_program():
    nc = bacc.Bacc("TRN2", target_bir_lowering=False, debug=False, num_devices=1)
    f32 = mybir.dt.float32
    f32r = mybir.dt.float32r
    ACT = mybir.ActivationFunctionType

    xx_d = nc.dram_tensor("xx", [128, PADN], f32r, kind="ExternalInput").ap()
    w0_d = nc.dram_tensor("w0", [64, NTAPS * 128], f32r, kind="ExternalInput").ap()
    w_d = nc.dram_tensor("w", [128, 3 * NTAPS * 128], f32r, kind="ExternalInput").ap()
    o_d = nc.dram_tensor("out", [C_OUT, L], f32, kind="ExternalOutput").ap()

    # x DMA slices: boundaries aligned so row tile r (rows 10r..10r+R+1,
    # i.e. cols < (10r+R+2)*50) is covered by the first slices.
    XS = (625, 1250, 1875, PADN)
    CS = (0, 834, 1667, PADN)  # slice bounds for elementwise / copies

    with tile.TileContext(nc) as tc:
        with (
            tc.tile_pool(name="big", bufs=1) as wpool,
            tc.tile_pool(name="outs", bufs=3) as opool,
            tc.tile_pool(name="psum", bufs=1, space="PSUM") as ppool,
        ):
            x_sb = wpool.tile([128, PADN], f32r, tag="xx")        # [x | x]
            t2 = wpool.tile([128, PADN], f32, tag="t2")          # [- | s]
            t3 = wpool.tile([128, PADN], f32, tag="t3")          # [- | s2]
            t23 = wpool.tile([128, PADN], f32, tag="t23")        # [- | s3]
            c1 = wpool.tile([128, PADN], f32r, tag="c1")         # [s | us]
            c2 = wpool.tile([128, PADN], f32r, tag="c2")         # [s2 | us2]
            c3 = wpool.tile([128, PADN], f32r, tag="c3")         # [s3 | us3]
            w0_sb = wpool.tile([128, NTAPS * 128], f32r, tag="w0")
            w_sb = wpool.tile([128, 3 * NTAPS * 128], f32r)
            warm = wpool.tile([128, 256], f32r, tag="warm")

            x_f32 = x_sb.bitcast(f32)
            c1f = c1.bitcast(f32)
            c2f = c2.bitcast(f32)
            c3f = c3.bitcast(f32)

            # ---- input DMAs (fine-sliced; each dma_start gets its own
            # hardware queue ~45GB/s, so slicing shortens the landing) ----
            # x lower half on sync, x upper half first on gpsimd: the two
            # halves of the first 625 cols stream in parallel.
            nc.sync.dma_start(out=x_sb[0:64, 0:625], in_=xx_d[0:64, 0:625])
            nc.gpsimd.dma_start(out=x_sb[64:128, 0:625], in_=xx_d[64:128, 0:625])
            for b in range(3):
                nc.sync.dma_start(out=x_sb[:, XS[b] : XS[b + 1]], in_=xx_d[:, XS[b] : XS[b + 1]])
            # scalar ring: chunk-0 weights in 3-tap pieces (lower half only;
            # the zero upper half is memset on-device), then chunk 1 (2 pcs).
            WB = NTAPS * 128
            for p in range(3):
                nc.scalar.dma_start(
                    out=w0_sb[0:64, p * 384 : (p + 1) * 384], in_=w0_d[:, p * 384 : (p + 1) * 384]
                )
            HWB = WB // 2
            nc.scalar.dma_start(out=w_sb[:, 0:HWB], in_=w_d[:, 0:HWB])
            nc.scalar.dma_start(out=w_sb[:, HWB:WB], in_=w_d[:, HWB:WB])
            # sync ring continues: w chunks 2-3 in halves
            for p in range(2, 6):
                nc.sync.dma_start(
                    out=w_sb[:, p * HWB : (p + 1) * HWB], in_=w_d[:, p * HWB : (p + 1) * HWB]
                )

            # gpsimd ring: PE warm tile + chunk-0 upper weights = 0
            nc.gpsimd.memset(warm.bitcast(f32)[:], 0.0)
            nc.gpsimd.memset(w0_sb.bitcast(f32)[64:128, :], 0.0)

            # ---- PE pre-warm while DMAs land (HAM/pstate ramp) ----
            warm_ps = ppool.tile([128, 256], f32, tag="warm_ps")
            for _ in range(N_WARM):
                nc.tensor.matmul(warm_ps[:], warm[:, 0:128], warm[:], start=True, stop=True)

            # ---- feature planes (half-partition ops; no [1|x] helper) ----
            # lower halves: s=x^2 in c1, s^2 in c2, s^3 in c3
            # upper halves: s,s^2,s^3 in t2/t3/t23, then *x -> c1/c2/c3
            LO = slice(0, 64)
            UP = slice(64, 128)
            for b in range(3):
                cs = slice(CS[b], CS[b + 1])
                nc.scalar.activation(c1[LO, cs], x_f32[LO, cs], ACT.Square)
                nc.scalar.activation(t2[UP, cs], x_f32[UP, cs], ACT.Square)
            for b in range(3):
                cs = slice(CS[b], CS[b + 1])
                nc.vector.tensor_mul(c1[UP, cs], t2[UP, cs], x_f32[UP, cs])
                nc.scalar.activation(c2[LO, cs], c1f[LO, cs], ACT.Square)
                nc.scalar.activation(t3[UP, cs], t2[UP, cs], ACT.Square)
            for b in range(3):
                cs = slice(CS[b], CS[b + 1])
                nc.vector.tensor_mul(c2[UP, cs], t3[UP, cs], x_f32[UP, cs])
                nc.gpsimd.tensor_mul(c3[LO, cs], c2f[LO, cs], c1f[LO, cs])
                nc.gpsimd.tensor_mul(t23[UP, cs], t3[UP, cs], t2[UP, cs])
            for b in range(3):
                cs = slice(CS[b], CS[b + 1])
                nc.vector.tensor_mul(c3[UP, cs], t23[UP, cs], x_f32[UP, cs])

            # ---- implicit GEMM: chunk-outer, tile-mid, tap-inner ----
            x_im = x_sb.rearrange("c (h w) -> c h w", h=HP)
            chunk_ims = [t.rearrange("c (h w) -> c h w", h=HP) for t in (c1, c2, c3)]
            psums = []
            h0s = []
            h0 = 0
            for it, R in enumerate(ROW_TILES):
                psums.append(ppool.tile([128, R * W], f32, name=f"ps{h0}", tag=f"ps{it}"))
                h0s.append(h0)
                h0 += R
            out_rings = (nc.sync, nc.gpsimd, nc.sync, nc.gpsimd)

            # chunk 0: [x|x] tile, upper-half weights zero
            for it, R in enumerate(ROW_TILES):
                h0 = h0s[it]
                for t9 in range(NTAPS):
                    dh, dw = t9 // K - 1, t9 % K - 1
                    r0 = h0 + dh + 1
                    nc.tensor.matmul(
                        psums[it][:],
                        w0_sb[:, t9 * 128 : (t9 + 1) * 128],
                        x_im[:, r0 : r0 + R, dw + 1 : dw + 1 + W],
                        start=(t9 == 0),
                        stop=False,
                    )
            # chunks 1-3 (K=128), staggered per-tile evacuation on the last
            for jj, im in enumerate(chunk_ims):
                for it, R in enumerate(ROW_TILES):
                    h0 = h0s[it]
                    for t9 in range(NTAPS):
                        dh, dw = t9 // K - 1, t9 % K - 1
                        r0 = h0 + dh + 1
                        nc.tensor.matmul(
                            psums[it][:],
                            w_sb[:, (jj * NTAPS + t9) * 128 : (jj * NTAPS + t9 + 1) * 128],
                            im[:, r0 : r0 + R, dw + 1 : dw + 1 + W],
                            start=False,
                            stop=(jj == 2 and t9 == NTAPS - 1),
                        )
                    if jj == 2:
                        # staggered evacuation: DVE PSUM->SBUF, then DMA out
                        o_sb = opool.tile([C_OUT, R * W], f32, tag="osb")
                        if it < len(ROW_TILES) - 1:
                            nc.vector.tensor_copy(o_sb[:], psums[it][:])
                            out_rings[it].dma_start(
                                out=o_d[:, h0 * W : (h0 + R) * W], in_=o_sb[:]
                            )
                        else:
                            # last tile: halve so the final DMA starts sooner
                            hn = R * W // 2
                            for hh, eng in ((0, nc.sync), (1, nc.gpsimd)):
                                nc.vector.tensor_copy(
                                    o_sb[:, hh * hn : (hh + 1) * hn],
                                    psums[it][:, hh * hn : (hh + 1) * hn],
                                )
                                eng.dma_start(
                                    out=o_d[:, h0 * W + hh * hn : h0 * W + (hh + 1) * hn],
                                    in_=o_sb[:, hh * hn : (hh + 1) * hn],
                                )

    nc.compile()
    return nc


def _host_prep(x, w_b, w_s, c):
    """Fold Hermite->monomial basis change, w_s, and a degree-7 polynomial
    fit of silu into the weights (fp64 host math)."""
    wb = w_b[..., 0].astype(np.float64)          # (O, 576)
    cw = (c[..., 0] * w_s[None, ..., 0]).astype(np.float64)  # (N, O, 576)

    # monomial weights for planes u^1..u^7 (+ constant -> bias)
    wm = np.zeros((8, C_OUT, C_IN * NTAPS), np.float64)
    wm[1] = 2 * cw[1] - 12 * cw[3] + 120 * cw[5] - 1680 * cw[7]
    wm[2] = 2 * cw[2] - 48 * cw[4] + 720 * cw[6]
    wm[3] = 8 * cw[3] - 160 * cw[5] + 3360 * cw[7]
    wm[4] = 16 * cw[4] - 480 * cw[6]
    wm[5] = 32 * cw[5] - 1344 * cw[7]
    wm[6] = 64 * cw[6]
    wm[7] = 128 * cw[7]
    bias = (cw[0] - 2 * cw[2] + 12 * cw[4] - 120 * cw[6]).sum(axis=1)  # (O,)

    # degree-7 LS fit of silu over the actual input values (+ Chebyshev
    # nodes over the input range for tail control), folded into wm/bias
    xs = np.asarray(x, np.float64).ravel()
    m = np.abs(xs).max() * 1.02
    nodes = m * np.cos(np.pi * (np.arange(2000) + 0.5) / 2000)
    fitx = np.concatenate([xs[::37], nodes, nodes, nodes])
    A = np.vander(fitx, 8, increasing=True)
    coef, *_ = np.linalg.lstsq(A, fitx / (1 + np.exp(-fitx)), rcond=None)
    for f in range(1, 8):
        wm[f] += coef[f] * wb
    bias = bias + coef[0] * wb.sum(axis=1)

    # chunk 0 (plane u, K=64): [k=64, tap=9, o=128]
    cidx = np.arange(C_IN)
    w0 = np.zeros((64, NTAPS, C_OUT), np.float32)
    for t in range(NTAPS):
        w0[:, t, :] = wm[1][:, cidx * NTAPS + t].T.astype(np.float32)
    # chunks 1-3: [k_part=128, chunk=3, tap=9, o=128]
    # chunk j, k_part = 64*half + c_in -> plane u^{2j+2+half}
    wl = np.zeros((128, 3, NTAPS, C_OUT), np.float32)
    for j in range(3):
        for half in range(2):
            f = 2 * j + 2 + half
            for t in range(NTAPS):
                wl[64 * half : 64 * (half + 1), j, t, :] = (
                    wm[f][:, cidx * NTAPS + t].T.astype(np.float32)
                )
    return (
        w0.reshape(64, NTAPS * 128),
        wl.reshape(128, 3 * NTAPS * 128),
        bias.astype(np.float32),
    )


def _prep_in_maps(x, w_b, w_s, c):
    w0, wl, bias = _host_prep(x, w_b, w_s, c)
    xi = np.asarray(x, np.float32)
    xp = np.zeros((B, C_IN, HP, WP), np.float32)
    xp[:, :, 1 : 1 + H, 1 : 1 + W] = xi
    xp = xp.reshape(B, C_IN, PADN)
    in_maps = []
    for i in range(B):
        xx = np.concatenate([xp[i], xp[i]], axis=0)        # [x | x]
        in_maps.append({"xx": xx, "w0": w0, "w": wl})
    return in_maps, bias


def kernel(x, w_b, w_s, c):
    if "nc" not in _CACHE:
        _CACHE["nc"] = _build_program()
    nc = _CACHE["nc"]

    in_maps, bias = _prep_in_maps(x, w_b, w_s, c)
    res = run_bass_kernel_spmd(nc, in_maps, core_ids=list(range(B)))
    out = np.stack([res.results[i]["out"] for i in range(B)], axis=0)
    out += bias[None, :, None]
    return out.reshape(B, C_OUT, H, W)
